# revision 9
# baseline (speedup 1.0000x reference)
"""Trainium2 Bass kernel: normalized min-sum LDPC decoder (nn_Decoding_model).

Sharding: pure batch data-parallelism. B=16 rows split across 8 NeuronCores
(2 rows/core); H-derived matrices are replicated per core.

Per core, per iteration (BL=2 batch rows):
  abc[b]   : [128, N] f16, each partition holds |so_b| (PE transpose of the
             column-major state + DRAM-broadcast DMA)
  nv       : per m-chunk [128, N] f16 = hbig(f16, NEG_BACK on non-edges) - abc
  vmax8    : DVE top-8 of nv per check row -> m1=-v0, m2=-v1 (dup-exact, so
             t2 = v0-v1 = 0 for tied minima automatically)
  osg      : ONE Act op: Sign(vm0 - nv) in {0 @ argmin(+ties), 1 else}
  parity   : P[m] = sum_n H^T[n,m]*(so[n]<0)  (fp8 matmul, exact counts)
  rs = 1-2*(P mod 2);  t1 = -rs*v0;  t2n = (v1-v0)*rs  (f16 rhs)
  main[n]  : sum_m H[m,n]*t1[m]          (f16 H stationary, fp32 PSUM)
  corr[n]  : sum_m (1-osg)[m,n]*t2[m] = -sum_m osg*t2n - sum_m t2n
             second term via a single (-1)-matrix matmul with rhs=sum_mc t2n
  so_new   : si + softplus(w)*sign(so)*(main+corr)

Layout "cm" = column-major [128, NC]: n = c*128 + p.
"""

from contextlib import ExitStack

import numpy as np

import concourse.bass as bass
import concourse.mybir as mybir
import concourse.tile as tile
from concourse import bacc, bass_utils

F32 = mybir.dt.float32
F16 = mybir.dt.float16
F8 = mybir.dt.float8e4
I32 = mybir.dt.int32
OP = mybir.AluOpType

NEG_BACK = -57344.0  # "minus infinity" for non-edges (f16-exact, dominates |so|)

B, M, N, IT = 16, 1024, 2048, 5
N_CORES = 8
BL = B // N_CORES


def build(nc: bass.Bass, M=M, N=N, BL=BL, IT=IT, nv_pool_tt=4, osg_dve=3, nv_bufs=6, osg_bufs=16):
    MC = M // 128  # m-chunks
    NC = N // 128  # n-chunks

    d_si = nc.dram_tensor("si_cm", [128, NC * BL], F32, kind="ExternalInput").ap()
    d_h = nc.dram_tensor("h_f16", [M, N], F16, kind="ExternalInput").ap()
    d_hb = nc.dram_tensor("h_big", [M, N], F16, kind="ExternalInput").ap()
    d_ht = nc.dram_tensor("h_t", [N, M], F8, kind="ExternalInput").ap()
    d_misc = nc.dram_tensor("misc_in", [128, 129], F32, kind="ExternalInput").ap()
    d_abc0 = nc.dram_tensor("abc0", [128, N * BL], F16, kind="ExternalInput").ap()
    d_out = nc.dram_tensor("out", [BL, N], F32, kind="ExternalOutput").ap()

    with tile.TileContext(nc) as tc, ExitStack() as ctx:
        const = ctx.enter_context(tc.tile_pool(name="const", bufs=1))
        state_p = ctx.enter_context(tc.tile_pool(name="state", bufs=2))
        negv_p = ctx.enter_context(tc.tile_pool(name="negv", bufs=nv_bufs))
        osg_p = ctx.enter_context(tc.tile_pool(name="osg", bufs=osg_bufs))
        psum_p = ctx.enter_context(tc.tile_pool(name="ps", bufs=1, space="PSUM"))
        pstr_p = ctx.enter_context(tc.tile_pool(name="pstr", bufs=2, space="PSUM"))
        dram_p = ctx.enter_context(tc.tile_pool(name="dram", bufs=1, space="DRAM"))

        # ---- persistent loads ----
        t_h = const.tile([128, MC * N], F16)  # H, m-chunk mc at cols [mc*N,(mc+1)*N)
        t_hb = const.tile([128, MC * N], F16)  # (1-H)*NEG_BACK
        t_ht = const.tile([128, NC * M], F8)  # H^T, n-chunk c at cols [c*M,(c+1)*M)
        misc = const.tile([128, 129], F32)  # ident(128) | norm(1)
        ident = misc[:, 0:128]
        t_norm = misc[:, 128:129]
        negones = const.tile([128, 128], F16)
        nc.vector.memset(negones[:], -1.0)
        nc.sync.dma_start(misc[:], d_misc)
        # dummy matmul so the PE observes misc's DMA sem before any transpose
        # (transpose-mode matmuls only support a single sync wait)
        pdum = pstr_p.tile([1, 1], F32, tag="dum", name="dum", bufs=1)
        nc.tensor.matmul(pdum[:], lhsT=ident[:, 0:1], rhs=ident[:, 0:1], start=True, stop=True)
        hb_r = d_hb.rearrange("(mc p) n -> mc p n", p=128)
        h_r = d_h.rearrange("(mc p) n -> mc p n", p=128)
        ht_r = d_ht.rearrange("(c p) m -> c p m", p=128)

        # small/urgent loads first: si (state derive) and abc0 (first nv),
        # then hb (nv mask), then ht (parity), then h (main matmuls, needed last)
        t_si = const.tile([128, NC * BL], F32)
        nc.sync.dma_start(t_si[:], d_si)
        si = [t_si[:, NC * b : NC * (b + 1)] for b in range(BL)]

        abc_p = ctx.enter_context(tc.tile_pool(name="abcp", bufs=2))
        abc = [None] * BL
        for b in range(BL):
            # iteration-0 abc comes precomputed from the host (f16)
            abc[b] = abc_p.tile([128, N], F16, tag=f"abc{b}", name=f"abc{b}")
            for q in range(2):
                nc.sync.dma_start(
                    abc[b][:, q * (N // 2) : (q + 1) * (N // 2)],
                    d_abc0[:, N * b + q * (N // 2) : N * b + (q + 1) * (N // 2)],
                )

        for mc in range(MC):
            nc.sync.dma_start(t_hb[:, mc * N : (mc + 1) * N], hb_r[mc])
        for c in range(NC):
            nc.sync.dma_start(t_ht[:, c * M : (c + 1) * M], ht_r[c])
        for mc in range(MC):
            nc.sync.dma_start(t_h[:, mc * N : (mc + 1) * N], h_r[mc])
        at_sb = [const.tile([NC, 128], F16, tag=f"atsb{b}", name=f"atsb{b}") for b in range(BL)]
        d_arow = dram_p.tile([BL, N], F16, name="d_arow")

        def derive_state(so_ap, b, negs):
            """From so (cm [128, NC]) write A(f32 |so|), S (f32 sign) and neg (fp8)."""
            st = state_p.tile([128, 2 * NC], F32, tag=f"st{b}", name=f"st{b}")
            A = st[:, 0:NC]
            S = st[:, NC : 2 * NC]
            nc.vector.tensor_scalar(
                out=A.bitcast(I32), in0=so_ap.bitcast(I32),
                scalar1=0x7FFFFFFF, scalar2=None, op0=OP.bitwise_and,
            )
            nc.vector.tensor_scalar(out=S, in0=so_ap, scalar1=0.0, scalar2=2.0, op0=OP.is_ge, op1=OP.mult)
            nc.vector.tensor_scalar(out=S, in0=S, scalar1=-1.0, scalar2=None, op0=OP.add)
            nc.vector.tensor_scalar(
                out=negs[:].rearrange("p (c two) -> p c two", two=2)[:, :, b : b + 1],
                in0=so_ap.unsqueeze(2),
                scalar1=0.0, scalar2=None, op0=OP.is_lt,
            )
            return A, S

        def bcast_A(A, b):
            """A (cm f32 [128, NC]) -> abc[b] [128, N] f16 row-major broadcast."""
            pt = pstr_p.tile([NC, 128], F32, tag="tr", name="tr")
            nc.tensor.transpose(pt[:], A, ident)
            nc.vector.tensor_copy(at_sb[b][:], pt[:])  # f32 psum -> f16 sbuf
            nc.sync.dma_start(d_arow[b : b + 1, :], at_sb[b][:])
            abc[b] = abc_p.tile([128, N], F16, tag=f"abc{b}", name=f"abc{b}")
            # split across DMA queues so one queue's bandwidth doesn't serialize
            for q in range(4):
                nc.sync.dma_start(
                    abc[b][:, q * (N // 4) : (q + 1) * (N // 4)],
                    d_arow[b : b + 1, q * (N // 4) : (q + 1) * (N // 4)].to_broadcast([128, N // 4]),
                )

        # ---- init state from si ----
        so = [si[b] for b in range(BL)]
        negs = state_p.tile([128, 2 * NC], F8, tag="negs", name="negs")
        AS = [derive_state(so[b], b, negs) for b in range(BL)]

        rs_of = {}
        negs_of = {0: negs}

        def parity(it):
            # P[m-part, 2*mc+b] = sum_n H^T * neg   (PE, early)
            pp = psum_p.tile([128, 2 * MC], F32, tag="pp", name="pp")
            for mc in range(MC):
                for c in range(NC):
                    nc.tensor.matmul(
                        pp[:, 2 * mc : 2 * mc + 2],
                        lhsT=t_ht[:, c * M + 128 * mc : c * M + 128 * (mc + 1)],
                        rhs=negs_of[it][:, 2 * c : 2 * c + 2],
                        start=(c == 0),
                        stop=(c == NC - 1),
                    )
            # rs in {-1, +1} from parity counts
            sm = state_p.tile([128, 2 * MC], F32, tag="sm", name="sm")
            rs = sm[:, 0 : 2 * MC]
            ri = state_p.tile([128, 2 * MC], I32, tag="ri", name="ri")
            nc.vector.tensor_copy(ri[:], pp[:])  # exact: P is integer-valued
            nc.vector.tensor_scalar(out=ri[:], in0=ri[:], scalar1=1, scalar2=None, op0=OP.bitwise_and)
            nc.vector.tensor_copy(rs, ri[:])
            nc.vector.tensor_scalar(out=rs, in0=rs, scalar1=-2.0, scalar2=1.0, op0=OP.mult, op1=OP.add)
            rs_of[it] = rs.rearrange("p (c two) -> p c two", two=2)

        def units(b, it):
            """nv / max8 / osg for all m-chunks of batch row b."""
            vmax = state_p.tile([128, 8 * MC], F32, tag=f"vm{b}", name=f"vm{b}")
            osgs = {}
            for mc in range(MC):
                nv = negv_p.tile([128, N], F16, tag="nv", name="nv")
                tt_eng = nc.gpsimd if mc < nv_pool_tt else nc.vector
                tt_eng.tensor_tensor(
                    out=nv[:], in0=t_hb[:, mc * N : (mc + 1) * N], in1=abc[b][:], op=OP.subtract
                )
                nc.vector.max(out=vmax[:, 8 * mc : 8 * mc + 8], in_=nv[:])
                osg = osg_p.tile([128, N], F16, tag="osg", name="osg")
                osgs[mc] = osg
                if mc < osg_dve:
                    nc.vector.tensor_scalar(
                        out=osg[:], in0=nv[:],
                        scalar1=vmax[:, 8 * mc : 8 * mc + 1], scalar2=None, op0=OP.is_lt,
                    )
                else:
                    # osg = Sign(vm0 - nv) in {0 @ argmin(+ties), 1 else}
                    nc.scalar.activation(
                        osg[:], nv[:], mybir.ActivationFunctionType.Sign,
                        bias=vmax[:, 8 * mc : 8 * mc + 1], scale=-1.0,
                    )
            return vmax, osgs

        def finish(b, it, vmax, osgs):
            """smalls, main/corr matmuls, combine, and state derive + bcast."""
            vm8 = vmax[:].rearrange("p (c k) -> p c k", k=8)
            rs_bv = rs_of[it]
            # smalls: t1 = -rs*vm0 ; t2n = (vm1-vm0)*rs ; t2nsum = sum_mc t2n
            tsm = state_p.tile([128, 2 * MC + 1], F16, tag=f"tsm{b}", name=f"tsm{b}")
            t1 = tsm[:, 0:MC]
            t2n = tsm[:, MC : 2 * MC]
            t2nsum = tsm[:, 2 * MC : 2 * MC + 1]
            nc.vector.scalar_tensor_tensor(
                out=t1.unsqueeze(2), in0=vm8[:, :, 0:1], scalar=-1.0,
                in1=rs_bv[:, :, b : b + 1], op0=OP.mult, op1=OP.mult,
            )
            dd = state_p.tile([128, MC], F32, tag=f"dd{b}", name=f"dd{b}")
            nc.vector.tensor_tensor(
                out=dd[:].unsqueeze(2), in0=vm8[:, :, 1:2], in1=vm8[:, :, 0:1], op=OP.subtract
            )
            nc.vector.tensor_tensor(
                out=t2n.unsqueeze(2), in0=dd[:].unsqueeze(2),
                in1=rs_bv[:, :, b : b + 1], op=OP.mult,
            )
            with nc.allow_low_precision(reason="sum of 8 f16 values, f16 range fine"):
                nc.vector.tensor_reduce(
                    out=t2nsum, in_=t2n, axis=mybir.AxisListType.X, op=OP.add
                )

            # main(b): pm[n-part, c] = sum_m H*t1
            pm = psum_p.tile([128, NC], F32, tag=f"pm{b}", name=f"pm{b}")
            for c in range(NC):
                for mc in range(MC):
                    nc.tensor.matmul(
                        pm[:, c : c + 1],
                        lhsT=t_h[:, mc * N + 128 * c : mc * N + 128 * (c + 1)],
                        rhs=t1[:, mc : mc + 1],
                        start=(mc == 0),
                        stop=(mc == MC - 1),
                    )
            # corr(b): pcr[n, c] = sum_m osg*t2n + (-1)*sum_m t2n = sum_m (1-osg)*t2
            pcr = psum_p.tile([128, NC], F32, tag=f"pcr{b}", name=f"pcr{b}")
            for c in range(NC):
                nc.tensor.matmul(
                    pcr[:, c : c + 1], lhsT=negones[:], rhs=t2nsum,
                    start=True, stop=False,
                )
                for mc in range(MC):
                    nc.tensor.matmul(
                        pcr[:, c : c + 1],
                        lhsT=osgs[mc][:, 128 * c : 128 * (c + 1)],
                        rhs=t2n[:, mc : mc + 1],
                        start=False,
                        stop=(mc == MC - 1),
                    )

            # combine(b): so_new = si + norm*S*(pm + pcr)
            A_old, S_old = AS[b]
            st2 = state_p.tile([128, 2 * NC], F32, tag=f"cmb{b}", name=f"cmb{b}")
            c1, so_n = st2[:, 0:NC], st2[:, NC : 2 * NC]
            nc.vector.tensor_copy(c1, pm[:])
            nc.vector.tensor_tensor(out=c1, in0=c1, in1=pcr[:], op=OP.add)
            nc.vector.scalar_tensor_tensor(
                out=c1, in0=c1, scalar=t_norm, in1=S_old, op0=OP.mult, op1=OP.mult
            )
            nc.vector.tensor_tensor(out=so_n, in0=si[b], in1=c1, op=OP.add)
            so[b] = so_n
            if it < IT - 1:
                if b == 0:
                    negs_of[it + 1] = state_p.tile([128, 2 * NC], F8, tag="negs", name="negs")
                A_n, S_n = derive_state(so_n, b, negs_of[it + 1])
                AS[b] = (A_n, S_n)
                bcast_A(A_n, b)

        # ---- software-pipelined iteration loop ----
        # Emission order keeps every in-order engine queue supplied with a
        # ready batch of unit work while the combine->bcast boundary chain of
        # the other batch row drains.
        parity(0)
        u0 = units(0, 0)
        for it in range(IT):
            u1 = units(1, it)
            finish(0, it, *u0)
            if it < IT - 1:
                u0 = units(0, it + 1)
            finish(1, it, *u1)
            if it < IT - 1:
                parity(it + 1)

        # output: so (cm) -> row-major [BL, N]
        out_sb = const.tile([NC, 256], F32, tag="outsb", name="outsb")
        for b in range(BL):
            po = pstr_p.tile([NC, 128], F32, tag="tr", name="tr")
            nc.tensor.transpose(po[:], so[b], ident)
            nc.vector.tensor_copy(out_sb[:, 128 * b : 128 * (b + 1)], po[:])
            nc.sync.dma_start(d_out[b : b + 1, :], out_sb[:, 128 * b : 128 * (b + 1)])

    return nc


_CACHE = {}


def _get_nc():
    if "nc" not in _CACHE:
        nc = bacc.Bacc("TRN2", target_bir_lowering=False)
        build(nc)
        nc.compile()
        _CACHE["nc"] = nc
    return _CACHE["nc"]


def _cm(row, ncnk):  # [N] -> [128, ncnk] column-major
    return row.reshape(ncnk, 128).T


def kernel(soft_input, H, labels, w):
    del labels  # unused by the reference computation
    soft_input = np.asarray(soft_input, dtype=np.float32)
    H = np.asarray(H)
    w = np.asarray(w, dtype=np.float32)
    NC = N // 128

    norm = np.log1p(np.exp(np.float64(w[0]))).astype(np.float32)
    f8 = mybir.dt.np(F8)
    f16 = mybir.dt.np(F16)
    h_f16 = H.astype(f16)
    h_big = ((1 - H) * NEG_BACK).astype(f16)
    h_t = np.ascontiguousarray(H.T).astype(f8)
    misc_in = np.concatenate(
        [np.eye(128, dtype=np.float32), np.full((128, 1), norm, dtype=np.float32)], axis=1
    )

    in_maps = []
    for core in range(N_CORES):
        rows = soft_input[BL * core : BL * (core + 1)]
        si_cm = np.concatenate([_cm(rows[b], NC) for b in range(BL)], axis=1)
        abc0 = np.concatenate(
            [np.broadcast_to(np.abs(rows[b]).astype(f16)[None, :], (128, N)) for b in range(BL)],
            axis=1,
        )
        in_maps.append(
            {
                "si_cm": np.ascontiguousarray(si_cm, dtype=np.float32),
                "h_f16": h_f16,
                "h_big": h_big,
                "h_t": h_t,
                "misc_in": misc_in,
                "abc0": np.ascontiguousarray(abc0, dtype=f16),
            }
        )

    nc = _get_nc()
    res = bass_utils.run_bass_kernel_spmd(nc, in_maps, core_ids=list(range(N_CORES)))
    out = np.concatenate([r["out"] for r in res.results], axis=0)
    return out.astype(np.float32)
